# revision 20
# baseline (speedup 1.0000x reference)
"""CfC (closed-form continuous-time) RNN kernel for Trainium2 (Bass/Tile).

Model (per timestep, ts=1.0):
    z  = [x_t, h]                                 # [B, I+U]
    g  = 1.7159*tanh(0.666*(z @ W_bb + b_bb))     # [B, BB]
    ff1 = tanh(g @ W_ff1 + b_ff1)                 # [B, U]
    ff2 = tanh(g @ W_ff2 + b_ff2)
    t_i = sigmoid(g@W_ta+b_ta + g@W_tb+b_tb)
    h   = ff1 + t_i*(ff2 - ff1)
    out_t = h @ W_fc + b_fc

Mapping: 8-way data parallel over batch (32 per core), feature-major layout
(features on SBUF/PSUM partitions, batch on the free dim) so per-partition
activation biases work and no transposes are needed inside the step.

Folds / restructurings:
  - sigmoid(w) = 0.5*(1+tanh(0.5*w)); the 0.5 pre-scaled into W_ta+W_tb.
  - 1.7159 (lecun_tanh gain) folded into the gate weight matrix.
  - 0.666 via the activation instruction's scale immediate; b_bb pre-scaled.
  - gate biases injected into PSUM via a K=6 indicator matmul.
  - h is never materialized: h = ff1 + 0.5*p with p = (1+th)*(ff2-ff1).
    Linear consumers (backbone matmul, readout) read ff1 and p separately,
    using 0.5-scaled copies of the weights for the p part. This removes one
    vector op + one sync hop from the serial chain and lets the backbone
    matmul start on the ff1 part right after the gate activation.
  - readout accumulates into PSUM incrementally each step (column slice per
    step), copied out + DMA'd once per R-step window.
  - x is transposed on the host to [I, T, B_local] for clean DMA.
"""

import numpy as np
from contextlib import ExitStack

B, I, U, O, BB = 256, 64, 256, 64, 128
NCORES = 8
BL = B // NCORES  # 32 batch per core

# dtypes (flip these for perf/precision tradeoffs)
import os as _os
WDT_NAME = _os.environ.get("CFC_WDT", "bfloat16")  # recurrent weight dtype
HDT_NAME = _os.environ.get("CFC_HDT", "bfloat16")  # gates / p dtype
XDT_NAME = _os.environ.get("CFC_XDT", "bfloat16")  # x / W_bbx dtype

_PROG_CACHE = {}
LAST_EXEC_NS = None
LAST_NC = None


def _build_program(T, R, XCH):
    import concourse.bacc as bacc
    import concourse.mybir as mybir
    import concourse.tile as tile

    F32 = mybir.dt.float32
    WDT = getattr(mybir.dt, WDT_NAME)
    HDT = getattr(mybir.dt, HDT_NAME)
    XDT = getattr(mybir.dt, XDT_NAME)
    AF = mybir.ActivationFunctionType
    OP = mybir.AluOpType

    nc = bacc.Bacc("TRN2", target_bir_lowering=False, debug=False,
                   num_devices=NCORES)

    # ---- DRAM I/O ----
    xT = nc.dram_tensor("xT", [I, T * BL], XDT, kind="ExternalInput")
    w1a = nc.dram_tensor("w1a", [128, 128], WDT, kind="ExternalInput")
    w1b = nc.dram_tensor("w1b", [128, 128], WDT, kind="ExternalInput")
    w1ah = nc.dram_tensor("w1ah", [128, 128], WDT, kind="ExternalInput")
    w1bh = nc.dram_tensor("w1bh", [128, 128], WDT, kind="ExternalInput")
    w1x = nc.dram_tensor("w1x", [I, 128], XDT, kind="ExternalInput")
    wcat = nc.dram_tensor("wcat", [128, 768], WDT, kind="ExternalInput")
    b6 = nc.dram_tensor("b6", [6, 128], WDT, kind="ExternalInput")
    ind = nc.dram_tensor("ind", [6, 6 * BL], WDT, kind="ExternalInput")
    wfca = nc.dram_tensor("wfca", [128, O], WDT, kind="ExternalInput")
    wfcb = nc.dram_tensor("wfcb", [128, O], WDT, kind="ExternalInput")
    wfcah = nc.dram_tensor("wfcah", [128, O], WDT, kind="ExternalInput")
    wfcbh = nc.dram_tensor("wfcbh", [128, O], WDT, kind="ExternalInput")
    bbb = nc.dram_tensor("bbb", [128, 1], F32, kind="ExternalInput")
    bfc = nc.dram_tensor("bfc", [O, 1], F32, kind="ExternalInput")
    outT = nc.dram_tensor("outT", [O, T * BL], F32, kind="ExternalOutput")
    hl = nc.dram_tensor("hl", [128, 2, BL], F32, kind="ExternalOutput")

    with ExitStack() as ctx:
        tc = ctx.enter_context(tile.TileContext(nc))

        cpool = ctx.enter_context(tc.tile_pool(name="consts", bufs=1))
        xpool = ctx.enter_context(tc.tile_pool(name="xchunks", bufs=3))
        gpool = ctx.enter_context(tc.tile_pool(name="g", bufs=3))
        sgpool = ctx.enter_context(tc.tile_pool(name="sg", bufs=3))
        dpool = ctx.enter_context(tc.tile_pool(name="dtile", bufs=3))
        ppool = ctx.enter_context(tc.tile_pool(name="ptile", bufs=3))
        stpool = ctx.enter_context(tc.tile_pool(name="stage", bufs=2))
        upool = ctx.enter_context(tc.tile_pool(name="upsum", bufs=2,
                                               space="PSUM"))
        vfpool = ctx.enter_context(tc.tile_pool(name="vfpsum", bufs=2,
                                                space="PSUM"))
        vtpool = ctx.enter_context(tc.tile_pool(name="vtpsum", bufs=2,
                                                space="PSUM"))
        ropool = ctx.enter_context(tc.tile_pool(name="ropsum", bufs=2,
                                                space="PSUM"))

        # ---- load constants ----
        def cload(name, dram, shape, dt):
            t = cpool.tile(shape, dt, name=name)
            nc.sync.dma_start(t[:], dram[:])
            return t

        w1a_s = cload("w1a_s", w1a, [128, 128], WDT)
        w1b_s = cload("w1b_s", w1b, [128, 128], WDT)
        w1ah_s = cload("w1ah_s", w1ah, [128, 128], WDT)
        w1bh_s = cload("w1bh_s", w1bh, [128, 128], WDT)
        w1x_s = cload("w1x_s", w1x, [I, 128], XDT)
        wcat_s = cload("wcat_s", wcat, [128, 768], WDT)
        b6_s = cload("b6_s", b6, [6, 128], WDT)
        ind_s = cload("ind_s", ind, [6, 6 * BL], WDT)
        wfca_s = cload("wfca_s", wfca, [128, O], WDT)
        wfcb_s = cload("wfcb_s", wfcb, [128, O], WDT)
        wfcah_s = cload("wfcah_s", wfcah, [128, O], WDT)
        wfcbh_s = cload("wfcbh_s", wfcbh, [128, O], WDT)
        bbb_s = cload("bbb_s", bbb, [128, 1], F32)
        bfc_s = cload("bfc_s", bfc, [O, 1], F32)

        # Pre-issue: u(0) x-part and v(0) biases before the loop.
        xt = xpool.tile([I, XCH * BL], XDT, name="xt")
        nc.sync.dma_start(xt[:], xT[:, 0:XCH * BL])
        u = upool.tile([128, BL], F32, name="u")
        nc.tensor.matmul(u[:], w1x_s[:], xt[:, 0:BL], start=True, stop=True)
        vf = vfpool.tile([128, 4 * BL], F32, name="vf")
        nc.tensor.matmul(vf[:], b6_s[:], ind_s[:, 0:4 * BL],
                         start=True, stop=False)
        vt = vtpool.tile([128, 2 * BL], F32, name="vt")
        nc.tensor.matmul(vt[:], b6_s[:], ind_s[:, 4 * BL:6 * BL],
                         start=True, stop=False)

        ro = None
        sg_last = p_last = None
        for t in range(T):
            u_cur, vf_cur, vt_cur = u, vf, vt

            # g~ = tanh(0.666*u + 0.666*b_bb)  (1.7159 gain lives in wcat)
            g = gpool.tile([128, BL], HDT, name="g")
            nc.scalar.activation(g[:], u_cur[:], AF.Tanh, bias=bbb_s[:, 0:1],
                                 scale=0.666)

            # ---- mm2: ff chunks first (ff-act gates on these 4 only) ----
            for c in range(4):
                nc.tensor.matmul(vf_cur[:, c * BL:(c + 1) * BL],
                                 wcat_s[:, c * 128:(c + 1) * 128], g[:],
                                 start=False, stop=(c == 3),
                                 skip_group_check=True)
            for c in range(2):
                nc.tensor.matmul(vt_cur[:, c * BL:(c + 1) * BL],
                                 wcat_s[:, (c + 4) * 128:(c + 5) * 128], g[:],
                                 start=False, stop=(c == 1),
                                 skip_group_check=True)

            # ---- pre-issue h-independent work for step t+1 ----
            last = t + 1 >= T
            if not last:
                if (t + 1) % XCH == 0:
                    xt = xpool.tile([I, XCH * BL], XDT, name="xt")
                    nc.sync.dma_start(
                        xt[:], xT[:, (t + 1) * BL:(t + 1 + XCH) * BL])
                co = (t + 1) % XCH
                u = upool.tile([128, BL], F32, name="u")
                nc.tensor.matmul(u[:], w1x_s[:],
                                 xt[:, co * BL:(co + 1) * BL],
                                 start=True, stop=False)
                vf = vfpool.tile([128, 4 * BL], F32, name="vf")
                nc.tensor.matmul(vf[:], b6_s[:], ind_s[:, 0:4 * BL],
                                 start=True, stop=False)
                vt = vtpool.tile([128, 2 * BL], F32, name="vt")
                nc.tensor.matmul(vt[:], b6_s[:], ind_s[:, 4 * BL:6 * BL],
                                 start=True, stop=False)

            # gates part 1: [ff1a ff1b ff2a ff2b] = tanh(vf)
            sg = sgpool.tile([128, 6 * BL], HDT, name="sg")
            nc.scalar.activation(sg[:, 0:4 * BL], vf_cur[:], AF.Tanh)

            # backbone + readout on the ff1 part (available immediately)
            if not last:
                nc.tensor.matmul(u[:], w1a_s[:], sg[:, 0:BL],
                                 start=False, stop=False,
                                 skip_group_check=True)
                nc.tensor.matmul(u[:], w1b_s[:], sg[:, BL:2 * BL],
                                 start=False, stop=False,
                                 skip_group_check=True)
            r = t % R
            if r == 0:
                ro = ropool.tile([O, R * BL], F32, name="ro")
            rs = ro[:, r * BL:(r + 1) * BL]
            nc.tensor.matmul(rs, wfca_s[:], sg[:, 0:BL],
                             start=True, stop=False, skip_group_check=True)
            nc.tensor.matmul(rs, wfcb_s[:], sg[:, BL:2 * BL],
                             start=False, stop=False, skip_group_check=True)

            # gates part 2: [tha thb] = tanh(vt)  (runs on ACT while DVE
            # computes d from part 1)
            nc.scalar.activation(sg[:, 4 * BL:6 * BL], vt_cur[:], AF.Tanh)

            # ---- p = (1+th)*(ff2-ff1) ----
            d = dpool.tile([128, 2 * BL], HDT, name="d")
            nc.vector.tensor_sub(d[:], sg[:, 2 * BL:4 * BL], sg[:, 0:2 * BL])
            p = ppool.tile([128, 2 * BL], HDT, name="p")
            nc.vector.scalar_tensor_tensor(p[:], sg[:, 4 * BL:6 * BL], 1.0,
                                           d[:], op0=OP.add, op1=OP.mult)

            # backbone + readout on the p part (0.5-scaled weights)
            if not last:
                nc.tensor.matmul(u[:], w1ah_s[:], p[:, 0:BL],
                                 start=False, stop=False,
                                 skip_group_check=True)
                nc.tensor.matmul(u[:], w1bh_s[:], p[:, BL:2 * BL],
                                 start=False, stop=True,
                                 skip_group_check=True)
            nc.tensor.matmul(rs, wfcah_s[:], p[:, 0:BL],
                             start=False, stop=False, skip_group_check=True)
            nc.tensor.matmul(rs, wfcbh_s[:], p[:, BL:2 * BL],
                             start=False, stop=(r == R - 1),
                             skip_group_check=True)

            # ---- stage + DMA per window ----
            if r == R - 1:
                st = stpool.tile([O, R * BL], F32, name="st")
                nc.vector.tensor_scalar_add(st[:], ro[:], bfc_s[:, 0:1])
                nc.sync.dma_start(outT[:, (t - R + 1) * BL:(t + 1) * BL],
                                  st[:])
            sg_last, p_last = sg, p

        # ---- h_last = ff1 + 0.5*p ----
        hls = stpool.tile([128, 2, BL], F32, name="hls")
        nc.vector.scalar_tensor_tensor(hls[:], p_last[:], 0.5,
                                       sg_last[:, 0:2 * BL],
                                       op0=OP.mult, op1=OP.add)
        nc.sync.dma_start(hl[:], hls[:])

    nc.compile()
    return nc


def _np_dt(name):
    import ml_dtypes
    return {"float32": np.float32, "float32r": np.float32,
            "bfloat16": ml_dtypes.bfloat16}[name]


def kernel(**inputs):
    from concourse.bass_utils import run_bass_kernel_spmd

    x = np.asarray(inputs["x"], np.float32)
    _, T, _ = x.shape
    R = 16 if T % 16 == 0 else T
    XCH = 64 if T % 64 == 0 else T

    key = (T, R, XCH, WDT_NAME, HDT_NAME, XDT_NAME)
    if key not in _PROG_CACHE:
        _PROG_CACHE[key] = _build_program(T, R, XCH)
    nc = _PROG_CACHE[key]
    global LAST_NC
    LAST_NC = nc

    wdt = _np_dt(WDT_NAME)
    W_bb = np.asarray(inputs["W_bb"], np.float32)
    Wt = 0.5 * (np.asarray(inputs["W_ta"], np.float32)
                + np.asarray(inputs["W_tb"], np.float32))
    bt = 0.5 * (np.asarray(inputs["b_ta"], np.float32)
                + np.asarray(inputs["b_tb"], np.float32))
    W_ff1 = np.asarray(inputs["W_ff1"], np.float32)
    W_ff2 = np.asarray(inputs["W_ff2"], np.float32)
    wcat = 1.7159 * np.concatenate(
        [W_ff1[:, :128], W_ff1[:, 128:], W_ff2[:, :128], W_ff2[:, 128:],
         Wt[:, :128], Wt[:, 128:]], axis=1)
    b_ff1 = np.asarray(inputs["b_ff1"], np.float32)
    b_ff2 = np.asarray(inputs["b_ff2"], np.float32)
    b6 = np.stack([b_ff1[:128], b_ff1[128:], b_ff2[:128], b_ff2[128:],
                   bt[:128], bt[128:]], axis=0)
    ind = np.zeros((6, 6 * BL), np.float32)
    for c in range(6):
        ind[c, c * BL:(c + 1) * BL] = 1.0
    W_fc = np.asarray(inputs["W_fc"], np.float32)

    xdt = _np_dt(XDT_NAME)
    shared = {
        "w1a": np.ascontiguousarray(W_bb[I:I + 128]).astype(wdt),
        "w1b": np.ascontiguousarray(W_bb[I + 128:I + 256]).astype(wdt),
        "w1ah": np.ascontiguousarray(0.5 * W_bb[I:I + 128]).astype(wdt),
        "w1bh": np.ascontiguousarray(0.5 * W_bb[I + 128:I + 256]).astype(wdt),
        "w1x": np.ascontiguousarray(W_bb[:I]).astype(xdt),
        "wcat": np.ascontiguousarray(wcat).astype(wdt),
        "b6": np.ascontiguousarray(b6).astype(wdt),
        "ind": ind.astype(wdt),
        "wfca": np.ascontiguousarray(W_fc[:128]).astype(wdt),
        "wfcb": np.ascontiguousarray(W_fc[128:]).astype(wdt),
        "wfcah": np.ascontiguousarray(0.5 * W_fc[:128]).astype(wdt),
        "wfcbh": np.ascontiguousarray(0.5 * W_fc[128:]).astype(wdt),
        "bbb": np.ascontiguousarray(
            (0.666 * np.asarray(inputs["b_bb"], np.float32))[:, None]),
        "bfc": np.ascontiguousarray(
            np.asarray(inputs["b_fc"], np.float32)[:, None]),
    }
    in_maps = []
    for c in range(NCORES):
        xc = x[c * BL:(c + 1) * BL]          # [BL, T, I]
        xTc = np.ascontiguousarray(
            np.transpose(xc, (2, 1, 0)).astype(xdt))  # [I, T, BL]
        in_maps.append({**shared, "xT": xTc.reshape(I, T * BL)})

    import os
    trace = bool(os.environ.get("CFC_TRACE"))
    res = run_bass_kernel_spmd(nc, in_maps, core_ids=list(range(NCORES)),
                               trace=trace)
    global LAST_EXEC_NS
    LAST_EXEC_NS = res.exec_time_ns

    out = np.empty((B, T, O), np.float32)
    h_last = np.empty((B, U), np.float32)
    for c in range(NCORES):
        oc = res.results[c]["outT"].reshape(O, T, BL)
        out[c * BL:(c + 1) * BL] = np.transpose(oc, (2, 1, 0))
        hc = res.results[c]["hl"].reshape(128, 2, BL)
        h_last[c * BL:(c + 1) * BL] = np.transpose(
            hc, (2, 1, 0)).reshape(BL, U)
    return out, h_last
